# revision 5
# baseline (speedup 1.0000x reference)
"""ConcatAttentionUnit (Bahdanau additive attention) Trainium2 Bass kernel.

Math (per batch b):
    sq = hq @ W1                  [Q=512, V=256]
    sp = hp @ W2                  [P=512, V=256]
    s[p,q]  = sum_v v[v] * tanh(sp[p,v] + sq[q,v])
    a = softmax_q(s); out = a @ hq            [P, 512]

Sharding: data-parallel over (batch, p-half): 8 cores, each handles one
(b, 256-row p slice). No collectives.

Per-core kernel strategy (all hot compute in [v-partition, *] layout):
  - transpose hq,hp via PE; project to sqT [v,q], spT [v,p] (bf16)
  - broadcast add sp+sq:  DVE tensor_scalar_add(in0=spT chunk [128,256p],
    scalar=sqT[:,q] column) -> tanh input tile, batched G queries per
    ACT Tanh instruction (amortizes ScalarE fixed overhead; ScalarE is
    the roofline engine here: B*P*Q*V/8 = 33.5M tanh elems/core)
  - v-dot: M=128/N=1 bf16 matmuls, lhsT = tanh tile [128v,128p],
    rhs = v chunk [128,1], accumulated into PSUM columns -> s[p,q]
  - exp via ACT with accum_out -> partition function Z for free
  - a @ hq as bf16 matmuls with lhsT = exp(s)^T (PE transpose), then
    per-partition scale by 1/Z at the end (softmax normalization folded
    into the epilogue).
"""

import numpy as np

import concourse.bass as bass  # noqa: F401  (registers rust bindings)
import concourse.mybir as mybir
import concourse.tile as tile
from concourse import bacc
from concourse.bass_utils import run_bass_kernel_spmd
from concourse.masks import make_identity

F32 = mybir.dt.float32
BF16 = mybir.dt.bfloat16
AF = mybir.ActivationFunctionType

B, Q, P, D, E, V = 4, 512, 512, 512, 512, 256
NCORES = 8
PSH = P * B // NCORES  # 256 p rows per core
QC = Q // 128  # q chunks
DC = D // 128  # d chunks
EC = E // 128  # e chunks
VC = V // 128  # v chunks
PC = PSH // 128  # p chunks
G = 16  # queries per ACT tanh instruction group
NG = Q // G


def kernel_body(nc, tc, hq, hp, w1, w2, vv, out):
    with (
        tc.tile_pool(name="persist", bufs=1) as pp,
        tc.tile_pool(name="tmp", bufs=1) as tp,
        tc.tile_pool(name="tanhbuf", bufs=2) as bp,
        tc.tile_pool(name="fin", bufs=2) as fin,
        tc.tile_pool(name="ps_s", bufs=1, space="PSUM") as ps_s_pool,
        tc.tile_pool(name="ps_w", bufs=2, space="PSUM") as ps_w,
        tc.tile_pool(name="ps_o", bufs=1, space="PSUM") as ps_o_pool,
    ):
        # ---------------- constants ----------------
        ident = pp.tile([128, 128], BF16, tag="ident")
        make_identity(nc, ident[:])
        v_f32 = tp.tile([128, VC], F32, tag="v_f32")
        for c in range(VC):
            nc.sync.dma_start(v_f32[:, c : c + 1], vv[c * 128 : (c + 1) * 128, :])
        v_bf = pp.tile([128, VC], BF16, tag="v_bf")
        nc.vector.tensor_copy(v_bf[:], v_f32[:])

        # ---------------- load + cast ----------------
        hq_f32 = tp.tile([128, QC * D], F32, tag="hq_f32")
        for qc in range(QC):
            nc.sync.dma_start(
                hq_f32[:, qc * D : (qc + 1) * D], hq[qc * 128 : (qc + 1) * 128, :]
            )
        hq_bf = pp.tile([128, QC * D], BF16, tag="hq_bf")
        nc.vector.tensor_copy(hq_bf[:], hq_f32[:])

        hp_f32 = tp.tile([128, PC * E], F32, tag="hp_f32")
        for pc in range(PC):
            nc.sync.dma_start(
                hp_f32[:, pc * E : (pc + 1) * E], hp[pc * 128 : (pc + 1) * 128, :]
            )
        hp_bf = tp.tile([128, PC * E], BF16, tag="hp_bf")
        nc.vector.tensor_copy(hp_bf[:], hp_f32[:])

        w1_f32 = tp.tile([128, DC * V], F32, tag="w1_f32")
        for dc in range(DC):
            nc.sync.dma_start(
                w1_f32[:, dc * V : (dc + 1) * V], w1[dc * 128 : (dc + 1) * 128, :]
            )
        w1_bf = tp.tile([128, DC * V], BF16, tag="w1_bf")
        nc.vector.tensor_copy(w1_bf[:], w1_f32[:])

        w2_f32 = tp.tile([128, EC * V], F32, tag="w2_f32")
        for ec in range(EC):
            nc.sync.dma_start(
                w2_f32[:, ec * V : (ec + 1) * V], w2[ec * 128 : (ec + 1) * 128, :]
            )
        w2_bf = tp.tile([128, EC * V], BF16, tag="w2_bf")
        nc.vector.tensor_copy(w2_bf[:], w2_f32[:])

        # ---------------- transposes (PE) ----------------
        # hqT[:, dc*Q + qc*128 + i] over partitions d  <-  hq[q, d]
        hqT = tp.tile([128, DC * Q], BF16, tag="hqT")
        for dc in range(DC):
            for qc in range(QC):
                ps = ps_w.tile([128, 128], BF16, tag="work")
                nc.tensor.transpose(
                    ps[:],
                    hq_bf[:, qc * D + dc * 128 : qc * D + (dc + 1) * 128],
                    ident[:],
                )
                nc.vector.tensor_copy(
                    hqT[:, dc * Q + qc * 128 : dc * Q + (qc + 1) * 128], ps[:]
                )
        hpT = tp.tile([128, EC * PSH], BF16, tag="hpT")
        for ec in range(EC):
            for pc in range(PC):
                ps = ps_w.tile([128, 128], BF16, tag="work")
                nc.tensor.transpose(
                    ps[:],
                    hp_bf[:, pc * E + ec * 128 : pc * E + (ec + 1) * 128],
                    ident[:],
                )
                nc.vector.tensor_copy(
                    hpT[:, ec * PSH + pc * 128 : ec * PSH + (pc + 1) * 128], ps[:]
                )

        # ---------------- projections ----------------
        # sqT[v, q] = sum_d W1[d, v] * hqT[d, q]
        sqT = pp.tile([128, VC * Q], F32, tag="sqT")
        for vc in range(VC):
            ps = ps_w.tile([128, 512], F32, tag="work")
            for dc in range(DC):
                nc.tensor.matmul(
                    ps[:, :Q],
                    w1_bf[:, dc * V + vc * 128 : dc * V + (vc + 1) * 128],
                    hqT[:, dc * Q : (dc + 1) * Q],
                    start=(dc == 0),
                    stop=(dc == DC - 1),
                )
            nc.vector.tensor_copy(sqT[:, vc * Q : (vc + 1) * Q], ps[:, :Q])
        # spT[v, p] = sum_e W2[e, v] * hpT[e, p]
        spT = pp.tile([128, VC * PSH], BF16, tag="spT")
        for vc in range(VC):
            ps = ps_w.tile([128, 512], F32, tag="work")
            for ec in range(EC):
                nc.tensor.matmul(
                    ps[:, :PSH],
                    w2_bf[:, ec * V + vc * 128 : ec * V + (vc + 1) * 128],
                    hpT[:, ec * PSH : (ec + 1) * PSH],
                    start=(ec == 0),
                    stop=(ec == EC - 1),
                )
            nc.vector.tensor_copy(spT[:, vc * PSH : (vc + 1) * PSH], ps[:, :PSH])

        # ---------------- main loop: tanh scores ----------------
        s_ps = [
            ps_s_pool.tile([128, Q], F32, tag=f"s{pc}", name=f"s_ps{pc}")
            for pc in range(PC)
        ]
        for grp in range(NG):
            buf = bp.tile([128, G * VC * PSH], BF16, tag="buf")
            for g in range(G):
                q = grp * G + g
                for vc in range(VC):
                    nc.vector.tensor_scalar_add(
                        buf[:, (g * VC + vc) * PSH : (g * VC + vc + 1) * PSH],
                        spT[:, vc * PSH : (vc + 1) * PSH],
                        sqT[:, vc * Q + q : vc * Q + q + 1],
                    )
            nc.scalar.activation(buf[:], buf[:], AF.Tanh)
            for g in range(G):
                q = grp * G + g
                for pc in range(PC):
                    for vc in range(VC):
                        off = (g * VC + vc) * PSH + pc * 128
                        nc.tensor.matmul(
                            s_ps[pc][:, q : q + 1],
                            buf[:, off : off + 128],
                            v_bf[:, vc : vc + 1],
                            start=(vc == 0),
                            stop=(vc == VC - 1),
                        )

        # ---------------- softmax (unnormalized) + output ----------------
        Zt = pp.tile([128, PC], F32, tag="Z")
        rec = pp.tile([128, PC], F32, tag="rec")
        exps = pp.tile([128, PC * Q], BF16, tag="exps")
        expT = pp.tile([128, PC * QC * 128], BF16, tag="expT")
        for pc in range(PC):
            nc.scalar.activation(
                exps[:, pc * Q : (pc + 1) * Q],
                s_ps[pc][:],
                AF.Exp,
                accum_out=Zt[:, pc : pc + 1],
            )
            nc.vector.reciprocal(rec[:, pc : pc + 1], Zt[:, pc : pc + 1])
            for qc in range(QC):
                ps = ps_w.tile([128, 128], BF16, tag="work")
                nc.tensor.transpose(
                    ps[:],
                    exps[:, pc * Q + qc * 128 : pc * Q + (qc + 1) * 128],
                    ident[:],
                )
                nc.vector.tensor_copy(
                    expT[:, (pc * QC + qc) * 128 : (pc * QC + qc + 1) * 128], ps[:]
                )
            o_ps = ps_o_pool.tile([128, D], F32, tag=f"o{pc}")
            for qc in range(QC):
                nc.tensor.matmul(
                    o_ps[:],
                    expT[:, (pc * QC + qc) * 128 : (pc * QC + qc + 1) * 128],
                    hq_bf[:, qc * D : (qc + 1) * D],
                    start=(qc == 0),
                    stop=(qc == QC - 1),
                )
            ob = fin.tile([128, D], F32, tag="ob")
            nc.vector.tensor_scalar_mul(ob[:], o_ps[:], rec[:, pc : pc + 1])
            nc.sync.dma_start(out[pc * 128 : (pc + 1) * 128, :], ob[:])


def build_program():
    nc = bacc.Bacc("TRN2", target_bir_lowering=False, debug=False)
    hq = nc.dram_tensor("hq_b", [Q, D], F32, kind="ExternalInput")
    hp = nc.dram_tensor("hp_s", [PSH, E], F32, kind="ExternalInput")
    w1 = nc.dram_tensor("W1", [D, V], F32, kind="ExternalInput")
    w2 = nc.dram_tensor("W2", [E, V], F32, kind="ExternalInput")
    vv = nc.dram_tensor("v", [V, 1], F32, kind="ExternalInput")
    out = nc.dram_tensor("out", [PSH, D], F32, kind="ExternalOutput")
    with tile.TileContext(nc) as tc:
        kernel_body(nc, tc, hq, hp, w1, w2, vv, out)
    nc.compile()
    return nc


_PROGRAM = None


def _get_program():
    global _PROGRAM
    if _PROGRAM is None:
        _PROGRAM = build_program()
    return _PROGRAM


def make_in_maps(hq, hp, W1, W2, v):
    w1 = np.ascontiguousarray(W1, dtype=np.float32)
    w2 = np.ascontiguousarray(W2, dtype=np.float32)
    vv = np.ascontiguousarray(v, dtype=np.float32).reshape(V, 1)
    in_maps = []
    for c in range(NCORES):
        b = c // (NCORES // B)
        ph = c % (NCORES // B)
        in_maps.append(
            {
                "hq_b": np.ascontiguousarray(hq[b], dtype=np.float32),
                "hp_s": np.ascontiguousarray(
                    hp[b, ph * PSH : (ph + 1) * PSH], dtype=np.float32
                ),
                "W1": w1,
                "W2": w2,
                "v": vv,
            }
        )
    return in_maps


def kernel(hq, hp, W1, W2, v, _trace=False, _return_raw=False, _tmpdir=None):
    nc = _get_program()
    in_maps = make_in_maps(hq, hp, W1, W2, v)
    res = run_bass_kernel_spmd(
        nc, in_maps, list(range(NCORES)), trace=_trace, tmpdir=_tmpdir
    )
    out = np.empty((B, P, D), dtype=np.float32)
    for c in range(NCORES):
        b = c // (NCORES // B)
        ph = c % (NCORES // B)
        out[b, ph * PSH : (ph + 1) * PSH, :] = res.results[c]["out"]
    if _return_raw:
        return out, res
    return out


# revision 13
# speedup vs baseline: 1.3403x; 1.3403x over previous
"""ConcatAttentionUnit (Bahdanau additive attention) Trainium2 Bass kernel.

Math (per batch b):
    sq = hq @ W1                  [Q=512, V=256]
    sp = hp @ W2                  [P=512, V=256]
    s[p,q]  = sum_v v[v] * tanh(sp[p,v] + sq[q,v])
    a = softmax_q(s); out = a @ hq            [P, 512]

Sharding: data-parallel over (batch, p-half): 8 cores, each handles one
(b, 256-row p slice). No collectives.

Per-core kernel strategy (all hot compute in [v-partition, *] layout):
  - transpose hq,hp via PE; project to sqT [v,q], spT [v,p] (bf16)
  - broadcast add sp+sq:  DVE tensor_scalar_add(in0=spT chunk [128,256p],
    scalar=sqT[:,q] column) -> tanh input tile, batched G queries per
    ACT Tanh instruction (amortizes ScalarE fixed overhead; ScalarE is
    the roofline engine here: B*P*Q*V/8 = 33.5M tanh elems/core)
  - v-dot: M=128/N=1 bf16 matmuls, lhsT = tanh tile [128v,128p],
    rhs = v chunk [128,1], accumulated into PSUM columns -> s[p,q]
  - exp via ACT with accum_out -> partition function Z for free
  - a @ hq as bf16 matmuls with lhsT = exp(s)^T (PE transpose), then
    per-partition scale by 1/Z at the end (softmax normalization folded
    into the epilogue).
"""

import numpy as np

import concourse.bass as bass  # noqa: F401  (registers rust bindings)
import concourse.mybir as mybir
import concourse.tile as tile
from concourse import bacc
from concourse.bass_utils import run_bass_kernel_spmd
from concourse.masks import make_identity

F32 = mybir.dt.float32
BF16 = mybir.dt.bfloat16
AF = mybir.ActivationFunctionType

B, Q, P, D, E, V = 4, 512, 512, 512, 512, 256
NCORES = 8
PSH = P * B // NCORES  # 256 p rows per core
QC = Q // 128  # q chunks
DC = D // 128  # d chunks
EC = E // 128  # e chunks
VC = V // 128  # v chunks
PC = PSH // 128  # p chunks
G = 16  # p rows per ACT tanh instruction group (FD = G*VC*Q = 16384)
NG = PSH // G


def kernel_body(nc, tc, hq, hp, w1, w2, vv, out):
    with (
        tc.tile_pool(name="persist", bufs=1) as pp,
        tc.tile_pool(name="tmp", bufs=1) as tp,
        tc.tile_pool(name="tanhbuf", bufs=2) as bp,
        tc.tile_pool(name="fin", bufs=2) as fin,
    ):
        # transposes + projections use 2 scratch PSUM banks, released before
        # the epilogue pools are opened (4 sT + 2 z + 2 o = 8 banks total).
        ps_w = tc.alloc_tile_pool(name="ps_w", bufs=2, space="PSUM")
        # ---------------- constants ----------------
        ident = pp.tile([128, 128], BF16, tag="ident")
        make_identity(nc, ident[:])
        v_f32 = tp.tile([128, VC], F32, tag="v_f32")
        for c in range(VC):
            nc.sync.dma_start(v_f32[:, c : c + 1], vv[c * 128 : (c + 1) * 128, :])
        v_bf = pp.tile([128, VC], BF16, tag="v_bf")
        nc.vector.tensor_copy(v_bf[:], v_f32[:])

        # ---------------- load + cast ----------------
        hq_f32 = tp.tile([128, QC * D], F32, tag="hq_f32")
        for qc in range(QC):
            nc.sync.dma_start(
                hq_f32[:, qc * D : (qc + 1) * D], hq[qc * 128 : (qc + 1) * 128, :]
            )
        hq_bf = pp.tile([128, QC * D], BF16, tag="hq_bf")
        nc.vector.tensor_copy(hq_bf[:], hq_f32[:])

        hp_f32 = tp.tile([128, PC * E], F32, tag="hp_f32")
        for pc in range(PC):
            nc.sync.dma_start(
                hp_f32[:, pc * E : (pc + 1) * E], hp[pc * 128 : (pc + 1) * 128, :]
            )
        hp_bf = tp.tile([128, PC * E], BF16, tag="hp_bf")
        nc.vector.tensor_copy(hp_bf[:], hp_f32[:])

        w1_f32 = tp.tile([128, DC * V], F32, tag="w1_f32")
        for dc in range(DC):
            nc.sync.dma_start(
                w1_f32[:, dc * V : (dc + 1) * V], w1[dc * 128 : (dc + 1) * 128, :]
            )
        w1_bf = tp.tile([128, DC * V], BF16, tag="w1_bf")
        nc.vector.tensor_copy(w1_bf[:], w1_f32[:])

        w2_f32 = tp.tile([128, EC * V], F32, tag="w2_f32")
        for ec in range(EC):
            nc.sync.dma_start(
                w2_f32[:, ec * V : (ec + 1) * V], w2[ec * 128 : (ec + 1) * 128, :]
            )
        w2_bf = tp.tile([128, EC * V], BF16, tag="w2_bf")
        nc.vector.tensor_copy(w2_bf[:], w2_f32[:])

        # ---------------- transposes (PE) ----------------
        # hqT[:, dc*Q + qc*128 + i] over partitions d  <-  hq[q, d]
        hqT = tp.tile([128, DC * Q], BF16, tag="hqT")
        for dc in range(DC):
            for qc in range(QC):
                ps = ps_w.tile([128, 128], BF16, tag="work")
                nc.tensor.transpose(
                    ps[:],
                    hq_bf[:, qc * D + dc * 128 : qc * D + (dc + 1) * 128],
                    ident[:],
                )
                nc.vector.tensor_copy(
                    hqT[:, dc * Q + qc * 128 : dc * Q + (qc + 1) * 128], ps[:]
                )
        hpT = tp.tile([128, EC * PSH], BF16, tag="hpT")
        for ec in range(EC):
            for pc in range(PC):
                ps = ps_w.tile([128, 128], BF16, tag="work")
                nc.tensor.transpose(
                    ps[:],
                    hp_bf[:, pc * E + ec * 128 : pc * E + (ec + 1) * 128],
                    ident[:],
                )
                nc.vector.tensor_copy(
                    hpT[:, ec * PSH + pc * 128 : ec * PSH + (pc + 1) * 128], ps[:]
                )

        # ---------------- projections ----------------
        # sqT[v, q] = sum_d W1[d, v] * hqT[d, q]   (bf16: the streamed operand)
        sqT = pp.tile([128, VC * Q], BF16, tag="sqT")
        for vc in range(VC):
            ps = ps_w.tile([128, 512], F32, tag="work")
            for dc in range(DC):
                nc.tensor.matmul(
                    ps[:, :Q],
                    w1_bf[:, dc * V + vc * 128 : dc * V + (vc + 1) * 128],
                    hqT[:, dc * Q : (dc + 1) * Q],
                    start=(dc == 0),
                    stop=(dc == DC - 1),
                )
            nc.vector.tensor_copy(sqT[:, vc * Q : (vc + 1) * Q], ps[:, :Q])
        # spT[v, p] = sum_e W2[e, v] * hpT[e, p]   (f32: the per-p scalar source)
        spT = pp.tile([128, VC * PSH], F32, tag="spT")
        for vc in range(VC):
            ps = ps_w.tile([128, 512], F32, tag="work")
            for ec in range(EC):
                nc.tensor.matmul(
                    ps[:, :PSH],
                    w2_bf[:, ec * V + vc * 128 : ec * V + (vc + 1) * 128],
                    hpT[:, ec * PSH : (ec + 1) * PSH],
                    start=(ec == 0),
                    stop=(ec == EC - 1),
                )
            nc.vector.tensor_copy(spT[:, vc * PSH : (vc + 1) * PSH], ps[:, :PSH])
        ones_bf = pp.tile([128, 1], BF16, tag="ones_bf")
        nc.vector.memset(ones_bf[:], 1.0)
        ps_w.release()
        ps_s_pool = tc.alloc_tile_pool(name="ps_s", bufs=1, space="PSUM")
        ps_o_pool = tc.alloc_tile_pool(name="ps_o", bufs=1, space="PSUM")
        ps_z_pool = tc.alloc_tile_pool(name="ps_z", bufs=1, space="PSUM")

        # ---------------- main loop: tanh scores, sT[q, p] in PSUM ----------
        # buf[v, q] = tanh(sqT[v, q] + spT[v, p]) for each p; the per-p value
        # rides the tensor_scalar per-partition operand (FD=512 amortizes the
        # ~128cyc scalar register load), tanh batched over G p's per ACT instr.
        sT_ps = [
            ps_s_pool.tile([128, PSH], F32, tag=f"sT{qc}", name=f"sT_ps{qc}")
            for qc in range(QC)
        ]
        for grp in range(NG):
            buf = bp.tile([128, G * VC * Q], BF16, tag="buf")
            for g in range(G):
                p = grp * G + g
                for vc in range(VC):
                    nc.vector.tensor_scalar_add(
                        buf[:, (g * VC + vc) * Q : (g * VC + vc + 1) * Q],
                        sqT[:, vc * Q : (vc + 1) * Q],
                        spT[:, vc * PSH + p : vc * PSH + p + 1],
                    )
            nc.scalar.activation(buf[:], buf[:], AF.Tanh)
            for g in range(G):
                p = grp * G + g
                for vc in range(VC):
                    for qc in range(QC):
                        off = (g * VC + vc) * Q + qc * 128
                        nc.tensor.matmul(
                            sT_ps[qc][:, p : p + 1],
                            buf[:, off : off + 128],
                            v_bf[:, vc : vc + 1],
                            start=(vc == 0),
                            stop=(vc == VC - 1),
                        )

        # ---------------- softmax (unnormalized) + output ----------------
        # exp(sT)[q, p] is directly the lhsT of the final matmul; Z[p] via
        # ones-matmuls (partition-dim sum); normalization folded into epilogue.
        exp_sT = pp.tile([128, QC * PSH], BF16, tag="exp_sT")
        for qc in range(QC):
            nc.scalar.activation(
                exp_sT[:, qc * PSH : (qc + 1) * PSH], sT_ps[qc][:], AF.Exp
            )
        rec = pp.tile([128, PC], F32, tag="rec")
        for pc in range(PC):
            z_ps = ps_z_pool.tile([128, 1], F32, tag=f"z{pc}", name=f"z_ps{pc}")
            for qc in range(QC):
                nc.tensor.matmul(
                    z_ps[:],
                    exp_sT[:, qc * PSH + pc * 128 : qc * PSH + (pc + 1) * 128],
                    ones_bf[:],
                    start=(qc == 0),
                    stop=(qc == QC - 1),
                )
            nc.vector.reciprocal(rec[:, pc : pc + 1], z_ps[:])
            o_ps = ps_o_pool.tile([128, D], F32, tag=f"o{pc}", name=f"o_ps{pc}")
            for qc in range(QC):
                nc.tensor.matmul(
                    o_ps[:],
                    exp_sT[:, qc * PSH + pc * 128 : qc * PSH + (pc + 1) * 128],
                    hq_bf[:, qc * D : (qc + 1) * D],
                    start=(qc == 0),
                    stop=(qc == QC - 1),
                )
            ob = fin.tile([128, D], F32, tag="ob")
            nc.vector.tensor_scalar_mul(ob[:], o_ps[:], rec[:, pc : pc + 1])
            nc.sync.dma_start(out[pc * 128 : (pc + 1) * 128, :], ob[:])
        ps_z_pool.release()
        ps_o_pool.release()
        ps_s_pool.release()


def build_program():
    nc = bacc.Bacc("TRN2", target_bir_lowering=False, debug=False)
    hq = nc.dram_tensor("hq_b", [Q, D], F32, kind="ExternalInput")
    hp = nc.dram_tensor("hp_s", [PSH, E], F32, kind="ExternalInput")
    w1 = nc.dram_tensor("W1", [D, V], F32, kind="ExternalInput")
    w2 = nc.dram_tensor("W2", [E, V], F32, kind="ExternalInput")
    vv = nc.dram_tensor("v", [V, 1], F32, kind="ExternalInput")
    out = nc.dram_tensor("out", [PSH, D], F32, kind="ExternalOutput")
    with tile.TileContext(nc) as tc:
        kernel_body(nc, tc, hq, hp, w1, w2, vv, out)
    nc.compile()
    return nc


_PROGRAM = None


def _get_program():
    global _PROGRAM
    if _PROGRAM is None:
        _PROGRAM = build_program()
    return _PROGRAM


def make_in_maps(hq, hp, W1, W2, v):
    w1 = np.ascontiguousarray(W1, dtype=np.float32)
    w2 = np.ascontiguousarray(W2, dtype=np.float32)
    vv = np.ascontiguousarray(v, dtype=np.float32).reshape(V, 1)
    in_maps = []
    for c in range(NCORES):
        b = c // (NCORES // B)
        ph = c % (NCORES // B)
        in_maps.append(
            {
                "hq_b": np.ascontiguousarray(hq[b], dtype=np.float32),
                "hp_s": np.ascontiguousarray(
                    hp[b, ph * PSH : (ph + 1) * PSH], dtype=np.float32
                ),
                "W1": w1,
                "W2": w2,
                "v": vv,
            }
        )
    return in_maps


def kernel(hq, hp, W1, W2, v, _trace=False, _return_raw=False, _tmpdir=None):
    nc = _get_program()
    in_maps = make_in_maps(hq, hp, W1, W2, v)
    res = run_bass_kernel_spmd(
        nc, in_maps, list(range(NCORES)), trace=_trace, tmpdir=_tmpdir
    )
    out = np.empty((B, P, D), dtype=np.float32)
    for c in range(NCORES):
        b = c // (NCORES // B)
        ph = c % (NCORES // B)
        out[b, ph * PSH : (ph + 1) * PSH, :] = res.results[c]["out"]
    if _return_raw:
        return out, res
    return out
